# revision 4
# baseline (speedup 1.0000x reference)
"""Multi-head attention layer (B=4, L=2048, D=1024, H=16) on 8 TRN2 NeuronCores.

Sharding: core c handles batch b = c//2 and heads [8*(c%2), 8*(c%2)+8) —
batch-parallel x tensor-parallel over heads.  Host sums the two partial
outputs per batch and adds bv@Wo + bo (bk drops exactly by softmax shift
invariance).

Per-core dataflow (all matmul inputs bf16, fp32 accumulation):
  qT/kT = Wq/Wk_slice as stationary against xT  -> [512, 2048] (e on partitions)
  v     = x @ Wv_slice (+ones col per head)      -> [2048, 8*65]
  scores transposed: ST[s, l] chunks = kT_h stationary vs qT_h
  A = exp(ST/8) on ACT (fp32 in, bf16 out), [128, 1024] payloads
  V[l, 65] = A_chunk.T @ v_aug accumulation (ones col -> softmax denom in
  col 64); normalize via per-partition reciprocal+tensor_scalar (DVE);
  PE-transpose V pairs into VT; out_partial = VT.T @ Wo_slice.

Scheduling: blocks are pair-major (m outer, lt inner).  Only the pair-0
q/k projections run before the first block; the v and remaining q/k
projections stream into the early blocks as psum-tagged quanta so the
first exp starts ~40us in.  Each block emits the previous block's AV
sweep as one dense PE burst after score-group 3 (keeps ACT fed and the
PE HAM clock warm).  Score matmuls are issued h2-adjacent targeting the
two 64-partition PE row groups.
"""

import sys
from contextlib import ExitStack

for _p in ("/opt/trn_rl_repo", "/root/.axon_site/_ro/trn_rl_repo"):
    if _p not in sys.path:
        sys.path.append(_p)

import numpy as np
import ml_dtypes

import concourse.bass as bass
import concourse.mybir as mybir
import concourse.tile as tile
from concourse import bacc
from concourse.bass_utils import run_bass_kernel_spmd
from concourse.masks import make_identity

BF16 = mybir.dt.bfloat16
F8 = mybir.dt.float8e4
F32 = mybir.dt.float32
AF = mybir.ActivationFunctionType

B, L, D = 4, 2048, 1024
N_CORES = 8
DH = 512          # per-core head dims (8 heads x 64)
E = 64
SCALE = 0.125     # 1/sqrt(E)

KD = D // 128     # 8 contraction chunks for projections
NL = L // 512     # 4 l-tiles
NS = L // 128     # 16 s-chunks / l-subs


def build_attention_nc():
    nc = bacc.Bacc("TRN2", target_bir_lowering=False, debug=False)

    xT_d = nc.dram_tensor("xT", [D, L], BF16, kind="ExternalInput").ap()
    wq_d = nc.dram_tensor("wq", [D, DH], BF16, kind="ExternalInput").ap()
    wk_d = nc.dram_tensor("wk", [D, DH], BF16, kind="ExternalInput").ap()
    wv_d = nc.dram_tensor("wv", [D, DH], BF16, kind="ExternalInput").ap()
    wo_d = nc.dram_tensor("wo", [DH, D], BF16, kind="ExternalInput").ap()
    bq_d = nc.dram_tensor("bq", [DH, 1], F32, kind="ExternalInput").ap()
    out_d = nc.dram_tensor("out", [L, D], F32, kind="ExternalOutput").ap()

    with tile.TileContext(nc) as tc, ExitStack() as ctx:
        const_pool = ctx.enter_context(tc.tile_pool(name="const", bufs=1))
        w_pool = ctx.enter_context(tc.tile_pool(name="w", bufs=1))
        qk_pool = ctx.enter_context(tc.tile_pool(name="qk", bufs=1))
        v_pool = ctx.enter_context(tc.tile_pool(name="v", bufs=1))
        at_pool = ctx.enter_context(tc.tile_pool(name="at", bufs=14))
        vs_pool = ctx.enter_context(tc.tile_pool(name="vs", bufs=1))
        vt_pool = ctx.enter_context(tc.tile_pool(name="vt", bufs=8))
        rec_pool = ctx.enter_context(tc.tile_pool(name="rec", bufs=8))
        osb_pool = ctx.enter_context(tc.tile_pool(name="osb", bufs=2))

        st_ps = ctx.enter_context(tc.tile_pool(name="st_ps", bufs=1, space="PSUM"))
        av_ps = ctx.enter_context(tc.tile_pool(name="av_ps", bufs=1, space="PSUM"))
        tr_ps = ctx.enter_context(tc.tile_pool(name="tr_ps", bufs=1, space="PSUM"))
        out_ps = ctx.enter_context(tc.tile_pool(name="out_ps", bufs=1, space="PSUM"))
        phase1_ctx = ExitStack()
        xt_pool = phase1_ctx.enter_context(tc.tile_pool(name="xt", bufs=1))

        ident = const_pool.tile([128, 128], BF16, tag="ident", name="ident")
        make_identity(nc, ident[:])
        bq_sb = const_pool.tile([128, 4], F32, tag="bq", name="bq_sb")
        for m in range(4):
            nc.sync.dma_start(bq_sb[:, m : m + 1], bq_d[128 * m : 128 * m + 128, :])

        xt = []
        for i in range(KD):
            t = xt_pool.tile([128, L], BF16, tag=f"xt{i}", name=f"xt{i}")
            nc.sync.dma_start(t[:], xT_d[128 * i : 128 * i + 128, :])
            xt.append(t)
        wq, wk, wv = [], [], []
        for name, lst, dram in (("wq", wq, wq_d), ("wk", wk, wk_d), ("wv", wv, wv_d)):
            for i in range(KD):
                t = xt_pool.tile([128, DH], BF16, tag=f"{name}{i}", name=f"{name}{i}")
                nc.sync.dma_start(t[:], dram[128 * i : 128 * i + 128, :])
                lst.append(t)
        wo = []
        for p in range(DH // 128):
            t = w_pool.tile([128, D], BF16, tag=f"wo{p}", name=f"wo{p}")
            nc.sync.dma_start(t[:], wo_d[128 * p : 128 * p + 128, :])
            wo.append(t)

        qT = [qk_pool.tile([128, L], BF16, tag=f"qT{m}", name=f"qT{m}") for m in range(4)]
        kT = [qk_pool.tile([128, L], BF16, tag=f"kT{m}", name=f"kT{m}") for m in range(4)]

        def emit_qk_proj(m, which, n, tag):
            # one psum group (8 matmuls) of the q or k projection
            if tag in ("outp",):
                ps = out_ps.tile([128, 512], F32, tag=tag, name="proj")
            elif tag in ("tr",):
                ps = tr_ps.tile([128, 512], F32, tag=tag, name="proj")
            elif tag.startswith("av"):
                ps = av_ps.tile([128, 260], F32, tag=tag, name="proj")
            else:
                ps = st_ps.tile([128, 512], F32, tag=tag, name="proj")
            w_ = wq if which == "q" else wk
            for kd in range(KD):
                nc.tensor.matmul(
                    ps[:], w_[kd][:, 128 * m : 128 * m + 128],
                    xt[kd][:, 512 * n : 512 * n + 512],
                    start=(kd == 0), stop=(kd == KD - 1))
            if which == "q":
                nc.vector.tensor_scalar_add(
                    qT[m][:, 512 * n : 512 * n + 512], ps[:], bq_sb[:, m : m + 1])
            else:
                nc.vector.tensor_copy(kT[m][:, 512 * n : 512 * n + 512], ps[:])

        # ---- prologue: just qk pair 0 — everything else streams into the
        #      early blocks so the first exp starts as soon as possible ----
        rot = ["st0", "st1", "outp", "tr"]
        for n in range(NL):
            emit_qk_proj(0, "q", n, rot[n % 4])
        for n in range(NL):
            emit_qk_proj(0, "k", n, rot[(n + 2) % 4])

        v_aug = [None] * NS
        vrot = ["av0", "av1", "outp", "tr"]

        def emit_v_proj(s, tag):
            pool = av_ps if tag.startswith("av") else out_ps if tag == "outp" else tr_ps
            ps = pool.tile([128, 512], F32, tag=tag, name="proj")
            for kd in range(KD):
                nc.tensor.matmul(
                    ps[:], xt[kd][:, 128 * s : 128 * s + 128], wv[kd][:],
                    start=(kd == 0), stop=(kd == KD - 1))
            t = v_pool.tile([128, 520], F8, tag=f"v{s}", name=f"vaug{s}")
            t3 = t[:].rearrange("p (h e) -> p h e", h=8)
            nc.vector.tensor_copy(t3[:, :, 0:64], ps[:].rearrange("p (h e) -> p h e", h=8))
            nc.vector.memset(t3[:, :, 64:65], 1.0)
            v_aug[s] = t

        # ---- attention blocks, pair-major ----
        vstage = [vs_pool.tile([128, DH], BF16, tag=f"vs{ls}", name=f"vs{ls}") for ls in range(NS)]

        def emit_block(m, lt, prev, v_quanta=False):
            ats = {0: [], 1: []}
            for g in range(8):
                stp = {}
                for h2 in range(2):
                    stp[h2] = st_ps.tile([128, 1024], F32, tag=f"st{h2}", name=f"st{h2}")
                for c2 in range(2):
                    s = 2 * g + c2
                    for h2 in range(2):
                        p0 = 64 * h2
                        nc.tensor.matmul(
                            stp[h2][:, 512 * c2 : 512 * c2 + 512],
                            kT[m][p0 : p0 + 64, 128 * s : 128 * s + 128],
                            qT[m][p0 : p0 + 64, 512 * lt : 512 * lt + 512],
                            start=True, stop=True)
                for h2 in range(2):
                    at = at_pool.tile([128, 1024], F8, tag=f"at{h2}", name=f"at{h2}")
                    nc.scalar.activation(at[:], stp[h2][:], AF.Exp, scale=SCALE)
                    ats[h2].append(at)
                if g == 3 and prev is not None:
                    emit_av(*prev)
                if v_quanta:
                    for s2 in (2 * g, 2 * g + 1):
                        emit_v_proj(s2, vrot[s2 % 4])
            return ats

        def emit_av(m, lt, ats):
            # dense AV burst + normalize into vstage for block (m, lt)
            for h2 in range(2):
                h = 2 * m + h2
                avp = av_ps.tile([128, 260], F32, tag=f"av{h2}", name=f"av{h2}")
                for j in range(4):
                    for s in range(NS):
                        g, c2 = divmod(s, 2)
                        nc.tensor.matmul(
                            avp[:, 65 * j : 65 * j + 65],
                            ats[h2][g][:, 512 * c2 + 128 * j : 512 * c2 + 128 * j + 128],
                            v_aug[s][:, 65 * h : 65 * h + 65],
                            start=(s == 0), stop=(s == NS - 1))
                for j in range(4):
                    r = rec_pool.tile([128, 1], F32, tag="rec", name="rec")
                    nc.vector.reciprocal(r[:], avp[:, 65 * j + 64 : 65 * j + 65])
                    nc.vector.tensor_scalar_mul(
                        vstage[4 * lt + j][:, 64 * h : 64 * h + 64],
                        avp[:, 65 * j : 65 * j + 64], r[:])

        def emit_outproj(lt):
            for ls in range(4 * lt, 4 * lt + 4):
                vts = []
                for p in range(4):
                    tp = tr_ps.tile([128, 128], BF16, tag="tr", name="trp")
                    nc.tensor.transpose(tp[:], vstage[ls][:, 128 * p : 128 * p + 128], ident[:])
                    vt = vt_pool.tile([128, 128], BF16, tag="vt", name="vt")
                    nc.vector.tensor_copy(vt[:], tp[:])
                    vts.append(vt)
                osb = osb_pool.tile([128, D], F32, tag="osb", name="osb")
                for d2 in range(2):
                    op = out_ps.tile([128, 512], F32, tag="outp", name="outp")
                    for p in range(4):
                        nc.tensor.matmul(
                            op[:], vts[p][:], wo[p][:, 512 * d2 : 512 * d2 + 512],
                            start=(p == 0), stop=(p == 3))
                    nc.vector.tensor_copy(osb[:, 512 * d2 : 512 * d2 + 512], op[:])
                nc.sync.dma_start(out_d[128 * ls : 128 * ls + 128, :], osb[:])

        # remaining projections, two psum-groups per block while pair m runs
        proj_quanta = {m: [(m + 1, w, n) for w in ("q", "k") for n in range(NL)]
                       for m in range(3)}

        prev = None
        for m in range(4):
            for lt in range(NL):
                ats = emit_block(m, lt, prev, v_quanta=(m == 0 and lt == 0))
                if m < 3:
                    q = proj_quanta[m]
                    for qi in range(2):
                        if q:
                            pm, w, n = q.pop(0)
                            emit_qk_proj(pm, w, n, "outp" if qi == 0 else "tr")
                if m == 3 and lt > 0:
                    emit_outproj(lt - 1)
                prev = (m, lt, ats)
        emit_av(*prev)
        emit_outproj(3)
        phase1_ctx.close()

    nc.compile()
    return nc


_NC_CACHE = []


def _make_in_maps(inputs):
    x = np.asarray(inputs["x"], dtype=np.float32)
    Wq = np.asarray(inputs["Wq"], dtype=np.float32)
    Wk = np.asarray(inputs["Wk"], dtype=np.float32)
    Wv = np.asarray(inputs["Wv"], dtype=np.float32)
    Wo = np.asarray(inputs["Wo"], dtype=np.float32)
    bq = np.asarray(inputs["bq"], dtype=np.float32)
    bf = ml_dtypes.bfloat16
    in_maps = []
    for c in range(N_CORES):
        b, hh = divmod(c, 2)
        sl = slice(DH * hh, DH * hh + DH)
        in_maps.append({
            "xT": np.ascontiguousarray(x[b].T).astype(bf),
            "wq": np.ascontiguousarray(Wq[:, sl]).astype(bf),
            "wk": np.ascontiguousarray(Wk[:, sl]).astype(bf),
            "wv": np.ascontiguousarray(Wv[:, sl]).astype(bf),
            "wo": np.ascontiguousarray(Wo[sl, :]).astype(bf),
            "bq": np.ascontiguousarray(bq[sl]).reshape(DH, 1).astype(np.float32),
        })
    return in_maps


def kernel(x, Wq, bq, Wk, bk, Wv, bv, Wo, bo):
    x = np.asarray(x, dtype=np.float32)
    Wq = np.asarray(Wq, dtype=np.float32)
    Wk = np.asarray(Wk, dtype=np.float32)
    Wv = np.asarray(Wv, dtype=np.float32)
    Wo = np.asarray(Wo, dtype=np.float32)
    bq = np.asarray(bq, dtype=np.float32)
    bv = np.asarray(bv, dtype=np.float32)
    bo = np.asarray(bo, dtype=np.float32)

    if not _NC_CACHE:
        _NC_CACHE.append(build_attention_nc())
    nc = _NC_CACHE[0]

    in_maps = _make_in_maps(dict(x=x, Wq=Wq, bq=bq, Wk=Wk, Wv=Wv, Wo=Wo))

    res = run_bass_kernel_spmd(nc, in_maps, list(range(N_CORES)))
    parts = [res.results[c]["out"] for c in range(N_CORES)]
    out = np.stack([parts[2 * b] + parts[2 * b + 1] for b in range(B)])
    out += (bv @ Wo + bo)[None, None, :]
    return out.astype(np.float32)



# revision 8
# speedup vs baseline: 1.3387x; 1.3387x over previous
"""Multi-head attention layer (B=4, L=2048, D=1024, H=16) on 8 TRN2 NeuronCores.

Sharding: core c handles batch b = c//2 and heads [8*(c%2), 8*(c%2)+8) —
batch-parallel x tensor-parallel over heads.  Host sums the two partial
outputs per batch and adds bv@Wo + bo (bk drops exactly by softmax shift
invariance).

Per-core dataflow (all matmul inputs bf16, fp32 accumulation):
  qT/kT = Wq/Wk_slice as stationary against xT  -> [512, 2048] (e on partitions)
  v     = x @ Wv_slice (+ones col per head)      -> [2048, 8*65]
  scores: per (g, c2) one psum tile [128 s, (h2, 512 l)] written by an
  adjacent pair of K=64 matmuls targeting PE row groups 0-63/64-127 (they
  run concurrently).  A = exp on ACT, FD=1024 per instruction.
  AV: per (h2, j) part: 16-matmul accumulation (A^T chunk stationary,
  v_aug moving, N=65); ones col -> softmax denom in col 64; normalize via
  reciprocal+tensor_scalar (DVE); PE-transpose V into VT;
  out_partial = VT.T @ Wo_slice.

Scheduling: blocks pair-major (m outer, lt inner), one block = 8 score
groups g feeding 16 ACT exps of 1147ns — the ACT engine is the roofline
(~295us busy) and everything else is paced to hide under it.  Per block:
score pairs ping-pong 2 psum tags; the previous block's AV runs as 8
parts in g=2..5; projection quanta (2-bank rotation outp/tr) fill
g=0,1,6,7 of m<3 blocks; out-proj per-ls chains fill those slots in m=3
blocks.  Prologue is 6 proj groups on 6 distinct psum banks fed by
column-quartered xT DMAs.
"""

import sys
from contextlib import ExitStack

for _p in ("/opt/trn_rl_repo", "/root/.axon_site/_ro/trn_rl_repo"):
    if _p not in sys.path:
        sys.path.append(_p)

import numpy as np
import ml_dtypes

import concourse.bass as bass
import concourse.mybir as mybir
import concourse.tile as tile
from concourse import bacc
from concourse.bass_utils import run_bass_kernel_spmd
from concourse.masks import make_identity

BF16 = mybir.dt.bfloat16
F32 = mybir.dt.float32
AF = mybir.ActivationFunctionType

B, L, D = 4, 2048, 1024
N_CORES = 8
DH = 512          # per-core head dims (8 heads x 64)
E = 64
SCALE = 0.125     # 1/sqrt(E)

KD = D // 128     # 8 contraction chunks for projections
NL = L // 512     # 4 l-tiles
NS = L // 128     # 16 s-chunks / l-subs


def build_attention_nc():
    nc = bacc.Bacc("TRN2", target_bir_lowering=False, debug=False)

    xT_d = nc.dram_tensor("xT", [D, L], BF16, kind="ExternalInput").ap()
    wq_d = nc.dram_tensor("wq", [D, DH], BF16, kind="ExternalInput").ap()
    wk_d = nc.dram_tensor("wk", [D, DH], BF16, kind="ExternalInput").ap()
    wv_d = nc.dram_tensor("wv", [D, DH], BF16, kind="ExternalInput").ap()
    wo_d = nc.dram_tensor("wo", [DH, D], BF16, kind="ExternalInput").ap()
    bq_d = nc.dram_tensor("bq", [DH, 1], F32, kind="ExternalInput").ap()
    out_d = nc.dram_tensor("out", [L, D], F32, kind="ExternalOutput").ap()

    with tile.TileContext(nc) as tc, ExitStack() as ctx:
        const_pool = ctx.enter_context(tc.tile_pool(name="const", bufs=1))
        w_pool = ctx.enter_context(tc.tile_pool(name="w", bufs=1))
        qk_pool = ctx.enter_context(tc.tile_pool(name="qk", bufs=1))
        v_pool = ctx.enter_context(tc.tile_pool(name="v", bufs=1))
        at_pool = ctx.enter_context(tc.tile_pool(name="at", bufs=14))
        vs_pool = ctx.enter_context(tc.tile_pool(name="vs", bufs=1))
        vt_pool = ctx.enter_context(tc.tile_pool(name="vt", bufs=8))
        rec_pool = ctx.enter_context(tc.tile_pool(name="rec", bufs=8))
        osb_pool = ctx.enter_context(tc.tile_pool(name="osb", bufs=2))

        st_ps = ctx.enter_context(tc.tile_pool(name="st_ps", bufs=1, space="PSUM"))
        av_ps = ctx.enter_context(tc.tile_pool(name="av_ps", bufs=1, space="PSUM"))
        tr_ps = ctx.enter_context(tc.tile_pool(name="tr_ps", bufs=1, space="PSUM"))
        out_ps = ctx.enter_context(tc.tile_pool(name="out_ps", bufs=1, space="PSUM"))
        phase1_ctx = ExitStack()
        xt_pool = phase1_ctx.enter_context(tc.tile_pool(name="xt", bufs=1))

        ident = const_pool.tile([128, 128], BF16, tag="ident", name="ident")
        make_identity(nc, ident[:])
        bq_sb = const_pool.tile([128, 4], F32, tag="bq", name="bq_sb")
        for m in range(4):
            nc.sync.dma_start(bq_sb[:, m : m + 1], bq_d[128 * m : 128 * m + 128, :])

        # DMA order: wq/wk first, then xT by column quarters, then wv, wo —
        # so the prologue projections start as early as possible.
        wq, wk, wv = [], [], []
        xt = [xt_pool.tile([128, L], BF16, tag=f"xt{i}", name=f"xt{i}")
              for i in range(KD)]
        for name, lst, dram in (("wq", wq, wq_d), ("wk", wk, wk_d)):
            for i in range(KD):
                t = xt_pool.tile([128, DH], BF16, tag=f"{name}{i}", name=f"{name}{i}")
                nc.sync.dma_start(t[:], dram[128 * i : 128 * i + 128, :])
                lst.append(t)
        for q4 in range(4):
            for i in range(KD):
                nc.sync.dma_start(
                    xt[i][:, 512 * q4 : 512 * q4 + 512],
                    xT_d[128 * i : 128 * i + 128, 512 * q4 : 512 * q4 + 512])
        for i in range(KD):
            t = xt_pool.tile([128, DH], BF16, tag=f"wv{i}", name=f"wv{i}")
            nc.sync.dma_start(t[:], wv_d[128 * i : 128 * i + 128, :])
            wv.append(t)
        wo = []
        for p in range(DH // 128):
            t = w_pool.tile([128, D], BF16, tag=f"wo{p}", name=f"wo{p}")
            nc.sync.dma_start(t[:], wo_d[128 * p : 128 * p + 128, :])
            wo.append(t)

        qT = [qk_pool.tile([128, L], BF16, tag=f"qT{m}", name=f"qT{m}") for m in range(4)]
        kT = [qk_pool.tile([128, L], BF16, tag=f"kT{m}", name=f"kT{m}") for m in range(4)]

        PROJ_TILE = {"st0": (st_ps, [128, 512]), "st1": (st_ps, [128, 512]),
                     "outp": (out_ps, [128, 512]), "tr": (tr_ps, [128, 512]),
                     "av0": (av_ps, [128, 512]), "av1": (av_ps, [128, 512])}

        def emit_qk_proj(m, which, n, tag):
            # one psum group (8 matmuls) of the q or k projection
            pool, shape = PROJ_TILE[tag]
            ps = pool.tile(shape, F32, tag=tag, name="proj")
            w_ = wq if which == "q" else wk
            for kd in range(KD):
                nc.tensor.matmul(
                    ps[:], w_[kd][:, 128 * m : 128 * m + 128],
                    xt[kd][:, 512 * n : 512 * n + 512],
                    start=(kd == 0), stop=(kd == KD - 1))
            if which == "q":
                nc.vector.tensor_scalar_add(
                    qT[m][:, 512 * n : 512 * n + 512], ps[:], bq_sb[:, m : m + 1])
            else:
                nc.vector.tensor_copy(kT[m][:, 512 * n : 512 * n + 512], ps[:])

        v_aug = [None] * NS

        def emit_v_proj(s, tag):
            pool, shape = PROJ_TILE[tag]
            ps = pool.tile(shape, F32, tag=tag, name="proj")
            for kd in range(KD):
                nc.tensor.matmul(
                    ps[:], xt[kd][:, 128 * s : 128 * s + 128], wv[kd][:],
                    start=(kd == 0), stop=(kd == KD - 1))
            t = v_pool.tile([128, 520], BF16, tag=f"v{s}", name=f"vaug{s}")
            t3 = t[:].rearrange("p (h e) -> p h e", h=8)
            nc.vector.tensor_copy(t3[:, :, 0:64], ps[:].rearrange("p (h e) -> p h e", h=8))
            nc.vector.memset(t3[:, :, 64:65], 1.0)
            v_aug[s] = t

        # ---- prologue: the 6 groups needed by block (0,0) + q(0,1),
        #      on 6 distinct psum banks ----
        emit_qk_proj(0, "q", 0, "st0")
        emit_qk_proj(0, "q", 1, "st1")
        for n in range(NL):
            emit_qk_proj(0, "k", n, ["outp", "tr", "av0", "av1"][n])

        # qk-projection quanta: block (m,lt) -> list of (pm, which, n),
        # placed in the light g-slots (g=0,1,6,7), each before first use.
        quanta = {
            (0, 1): [(0, "q", 2), (0, "q", 3), (1, "k", 0), (1, "k", 1)],
            (0, 2): [(1, "k", 2), (1, "k", 3), (1, "q", 0), (1, "q", 1)],
            (0, 3): [(1, "q", 2), (1, "q", 3), (2, "k", 0), (2, "k", 1)],
            (1, 0): [(2, "k", 2), (2, "k", 3), (2, "q", 0), (2, "q", 1)],
            (1, 1): [(2, "q", 2), (2, "q", 3), (3, "k", 0), (3, "k", 1)],
            (1, 2): [(3, "k", 2), (3, "k", 3), (3, "q", 0), (3, "q", 1)],
            (1, 3): [(3, "q", 2), (3, "q", 3)],
        }

        # ---- attention blocks ----
        vstage = [vs_pool.tile([128, DH], BF16, tag=f"vs{ls}", name=f"vs{ls}") for ls in range(NS)]

        def emit_av_part(prev, part):
            # one (h2, j) slice of the AV sweep for block `prev`
            pm, plt, ats = prev
            h2, j = divmod(part, 4)
            h = 2 * pm + h2
            avp = av_ps.tile([128, 65], F32, tag=f"av{part % 2}", name="avp")
            for s in range(NS):
                nc.tensor.matmul(
                    avp[:], ats[s][:, 512 * h2 + 128 * j : 512 * h2 + 128 * j + 128],
                    v_aug[s][:, 65 * h : 65 * h + 65],
                    start=(s == 0), stop=(s == NS - 1))
            r = rec_pool.tile([128, 1], F32, tag="rec", name="rec")
            nc.vector.reciprocal(r[:], avp[:, 64:65])
            nc.vector.tensor_scalar_mul(
                vstage[4 * plt + j][:, 64 * h : 64 * h + 64], avp[:, 0:64], r[:])

        def emit_outproj_ls(ls):
            # per-ls out-projection chain: 4 PE transposes -> vt -> 2 psum
            # groups -> osb -> DMA
            vts = []
            for p in range(4):
                tp = tr_ps.tile([128, 128], BF16, tag="tr", name="trp")
                nc.tensor.transpose(tp[:], vstage[ls][:, 128 * p : 128 * p + 128], ident[:])
                vt = vt_pool.tile([128, 128], BF16, tag="vt", name="vt")
                nc.vector.tensor_copy(vt[:], tp[:])
                vts.append(vt)
            osb = osb_pool.tile([128, D], F32, tag="osb", name="osb")
            for d2 in range(2):
                op = out_ps.tile([128, 512], F32, tag="outp", name="outp")
                for p in range(4):
                    nc.tensor.matmul(
                        op[:], vts[p][:], wo[p][:, 512 * d2 : 512 * d2 + 512],
                        start=(p == 0), stop=(p == 3))
                nc.vector.tensor_copy(osb[:, 512 * d2 : 512 * d2 + 512], op[:])
            nc.sync.dma_start(out_d[128 * ls : 128 * ls + 128, :], osb[:])

        outproj_q = []

        def emit_block(m, lt, prev, outproj_new):
            # one (head-pair, l-tile) block: 8 score groups g; each g makes
            # one [128, 1024] psum tile per c2 (both h2 halves, adjacent
            # matmul pair -> concurrent PE row groups) and exps it.
            q = list(quanta.get((m, lt), []))
            vq = list(range(NS)) if (m, lt) == (0, 0) else []
            ats = [None] * NS
            for g in range(8):
                for c2 in range(2):
                    s = 2 * g + c2
                    stc = st_ps.tile([128, 1024], F32, tag=f"st{c2}", name=f"st{c2}")
                    for h2 in range(2):
                        p0 = 64 * h2
                        nc.tensor.matmul(
                            stc[:, 512 * h2 : 512 * h2 + 512],
                            kT[m][p0 : p0 + 64, 128 * s : 128 * s + 128],
                            qT[m][p0 : p0 + 64, 512 * lt : 512 * lt + 512],
                            start=True, stop=True)
                    at = at_pool.tile([128, 1024], BF16, tag=f"at{c2}", name=f"at{c2}")
                    nc.scalar.activation(at[:], stc[:], AF.Exp, scale=SCALE)
                    ats[s] = at
                if 2 <= g <= 5 and prev is not None:
                    emit_av_part(prev, 2 * (g - 2))
                    emit_av_part(prev, 2 * (g - 2) + 1)
                if g == 6:
                    # this block's lt-1 out-proj becomes legal once the AV
                    # parts above (g=2..5) have filled vstage
                    outproj_q.extend(outproj_new)
                    outproj_new = []
                if vq:
                    emit_v_proj(vq.pop(0), ["av0", "av1", "outp", "tr"][g % 4])
                    emit_v_proj(vq.pop(0), ["av1", "outp", "tr", "av0"][g % 4])
                elif g in (0, 1, 6, 7):
                    if q:
                        pm, w_, n = q.pop(0)
                        emit_qk_proj(pm, w_, n, "outp" if g in (0, 6) else "tr")
                    elif outproj_q:
                        emit_outproj_ls(outproj_q.pop(0))
            return ats

        prev = None
        for m in range(4):
            for lt in range(NL):
                new = [4 * (lt - 1) + i for i in range(4)] if (m == 3 and lt > 0) else []
                ats = emit_block(m, lt, prev, new)
                prev = (m, lt, ats)
        for ls in outproj_q:
            emit_outproj_ls(ls)
        for j in range(4):
            emit_av_part(prev, j)        # (h2=0, j)
            emit_av_part(prev, 4 + j)    # (h2=1, j)
            emit_outproj_ls(12 + j)
        phase1_ctx.close()

    nc.compile()
    return nc


_NC_CACHE = []


def _make_in_maps(inputs):
    x = np.asarray(inputs["x"], dtype=np.float32)
    Wq = np.asarray(inputs["Wq"], dtype=np.float32)
    Wk = np.asarray(inputs["Wk"], dtype=np.float32)
    Wv = np.asarray(inputs["Wv"], dtype=np.float32)
    Wo = np.asarray(inputs["Wo"], dtype=np.float32)
    bq = np.asarray(inputs["bq"], dtype=np.float32)
    bf = ml_dtypes.bfloat16
    in_maps = []
    for c in range(N_CORES):
        b, hh = divmod(c, 2)
        sl = slice(DH * hh, DH * hh + DH)
        in_maps.append({
            "xT": np.ascontiguousarray(x[b].T).astype(bf),
            "wq": np.ascontiguousarray(Wq[:, sl]).astype(bf),
            "wk": np.ascontiguousarray(Wk[:, sl]).astype(bf),
            "wv": np.ascontiguousarray(Wv[:, sl]).astype(bf),
            "wo": np.ascontiguousarray(Wo[sl, :]).astype(bf),
            "bq": np.ascontiguousarray(bq[sl]).reshape(DH, 1).astype(np.float32),
        })
    return in_maps


def kernel(x, Wq, bq, Wk, bk, Wv, bv, Wo, bo):
    x = np.asarray(x, dtype=np.float32)
    Wq = np.asarray(Wq, dtype=np.float32)
    Wk = np.asarray(Wk, dtype=np.float32)
    Wv = np.asarray(Wv, dtype=np.float32)
    Wo = np.asarray(Wo, dtype=np.float32)
    bq = np.asarray(bq, dtype=np.float32)
    bv = np.asarray(bv, dtype=np.float32)
    bo = np.asarray(bo, dtype=np.float32)

    if not _NC_CACHE:
        _NC_CACHE.append(build_attention_nc())
    nc = _NC_CACHE[0]

    in_maps = _make_in_maps(dict(x=x, Wq=Wq, bq=bq, Wk=Wk, Wv=Wv, Wo=Wo))

    res = run_bass_kernel_spmd(nc, in_maps, list(range(N_CORES)))
    parts = [res.results[c]["out"] for c in range(N_CORES)]
    out = np.stack([parts[2 * b] + parts[2 * b + 1] for b in range(B)])
    out += (bv @ Wo + bo)[None, None, :]
    return out.astype(np.float32)


# revision 12
# speedup vs baseline: 1.3527x; 1.0105x over previous
"""Multi-head attention layer (B=4, L=2048, D=1024, H=16) on 8 TRN2 NeuronCores.

Sharding: core c handles batch b = c//2 and heads [8*(c%2), 8*(c%2)+8) —
batch-parallel x tensor-parallel over heads.  Host sums the two partial
outputs per batch and adds bv@Wo + bo (bk drops exactly by softmax shift
invariance).

Per-core dataflow (all matmul inputs bf16, fp32 accumulation):
  qT/kT = Wq/Wk_slice as stationary against xT  -> [512, 2048] (e on partitions)
  v     = x @ Wv_slice (+ones col per head)      -> [2048, 8*65]
  scores: per (g, c2) one psum tile [128 s, (h2, 512 l)] written by an
  adjacent pair of K=64 matmuls targeting PE row groups 0-63/64-127 (they
  run concurrently).  A = exp on ACT, FD=1024 per instruction.
  AV: per (h2, j) part: 16-matmul accumulation (A^T chunk stationary,
  v_aug moving, N=65); ones col -> softmax denom in col 64; normalize via
  reciprocal+tensor_scalar (DVE); PE-transpose V into VT;
  out_partial = VT.T @ Wo_slice.

Scheduling: blocks pair-major (m outer, lt inner), one block = 8 score
groups g feeding 16 ACT exps of 1147ns — the ACT engine is the roofline
(~295us busy) and everything else is paced to hide under it.  Per block:
score pairs ping-pong 2 psum tags; the previous block's AV runs as 8
parts in g=2..5; projection quanta (2-bank rotation outp/tr) fill
g=0,1,6,7 of m<3 blocks; out-proj per-ls chains fill those slots in m=3
blocks.  Prologue is 6 proj groups on 6 distinct psum banks fed by
column-quartered xT DMAs.
"""

import sys
from contextlib import ExitStack

for _p in ("/opt/trn_rl_repo", "/root/.axon_site/_ro/trn_rl_repo"):
    if _p not in sys.path:
        sys.path.append(_p)

import numpy as np
import ml_dtypes

import concourse.bass as bass
import concourse.mybir as mybir
import concourse.tile as tile
from concourse import bacc
from concourse.bass_utils import run_bass_kernel_spmd
from concourse.masks import make_identity

BF16 = mybir.dt.bfloat16
F32 = mybir.dt.float32
AF = mybir.ActivationFunctionType

B, L, D = 4, 2048, 1024
N_CORES = 8
DH = 512          # per-core head dims (8 heads x 64)
E = 64
SCALE = 0.125     # 1/sqrt(E)

KD = D // 128     # 8 contraction chunks for projections
NL = L // 512     # 4 l-tiles
NS = L // 128     # 16 s-chunks / l-subs


def build_attention_nc():
    nc = bacc.Bacc("TRN2", target_bir_lowering=False, debug=False)

    xT_d = nc.dram_tensor("xT", [D, L], BF16, kind="ExternalInput").ap()
    wq_d = nc.dram_tensor("wq", [D, DH], BF16, kind="ExternalInput").ap()
    wk_d = nc.dram_tensor("wk", [D, DH], BF16, kind="ExternalInput").ap()
    wv_d = nc.dram_tensor("wv", [D, DH], BF16, kind="ExternalInput").ap()
    wo_d = nc.dram_tensor("wo", [DH, D], BF16, kind="ExternalInput").ap()
    bq_d = nc.dram_tensor("bq", [DH, 1], F32, kind="ExternalInput").ap()
    out_d = nc.dram_tensor("out", [L, D], F32, kind="ExternalOutput").ap()

    with tile.TileContext(nc) as tc, ExitStack() as ctx:
        const_pool = ctx.enter_context(tc.tile_pool(name="const", bufs=1))
        w_pool = ctx.enter_context(tc.tile_pool(name="w", bufs=1))
        qk_pool = ctx.enter_context(tc.tile_pool(name="qk", bufs=1))
        v_pool = ctx.enter_context(tc.tile_pool(name="v", bufs=1))
        at_pool = ctx.enter_context(tc.tile_pool(name="at", bufs=14))
        vs_pool = ctx.enter_context(tc.tile_pool(name="vs", bufs=1))
        vt_pool = ctx.enter_context(tc.tile_pool(name="vt", bufs=8))
        rec_pool = ctx.enter_context(tc.tile_pool(name="rec", bufs=8))
        osb_pool = ctx.enter_context(tc.tile_pool(name="osb", bufs=2))

        st_ps = ctx.enter_context(tc.tile_pool(name="st_ps", bufs=1, space="PSUM"))
        av_ps = ctx.enter_context(tc.tile_pool(name="av_ps", bufs=1, space="PSUM"))
        tr_ps = ctx.enter_context(tc.tile_pool(name="tr_ps", bufs=1, space="PSUM"))
        out_ps = ctx.enter_context(tc.tile_pool(name="out_ps", bufs=1, space="PSUM"))
        phase1_ctx = ExitStack()
        xt_pool = phase1_ctx.enter_context(tc.tile_pool(name="xt", bufs=1))

        ident = const_pool.tile([128, 128], BF16, tag="ident", name="ident")
        make_identity(nc, ident[:])
        bq_sb = const_pool.tile([128, 4], F32, tag="bq", name="bq_sb")
        for m in range(4):
            nc.sync.dma_start(bq_sb[:, m : m + 1], bq_d[128 * m : 128 * m + 128, :])

        # DMA order: wq/wk first, then xT by column quarters, then wv, wo —
        # so the prologue projections start as early as possible.
        wq, wk, wv = [], [], []
        xt = [xt_pool.tile([128, L], BF16, tag=f"xt{i}", name=f"xt{i}")
              for i in range(KD)]
        for name, lst, dram in (("wq", wq, wq_d), ("wk", wk, wk_d)):
            for i in range(KD):
                t = xt_pool.tile([128, DH], BF16, tag=f"{name}{i}", name=f"{name}{i}")
                nc.sync.dma_start(t[:], dram[128 * i : 128 * i + 128, :])
                lst.append(t)
        for i in range(KD):
            nc.sync.dma_start(xt[i][:, 0:1024], xT_d[128 * i : 128 * i + 128, 0:1024])
        for i in range(KD):
            t = xt_pool.tile([128, DH], BF16, tag=f"wv{i}", name=f"wv{i}")
            nc.sync.dma_start(t[:], wv_d[128 * i : 128 * i + 128, :])
            wv.append(t)
        for i in range(KD):
            nc.sync.dma_start(xt[i][:, 1024:2048], xT_d[128 * i : 128 * i + 128, 1024:2048])
        wo = []
        for p in range(DH // 128):
            t = w_pool.tile([128, D], BF16, tag=f"wo{p}", name=f"wo{p}")
            nc.sync.dma_start(t[:], wo_d[128 * p : 128 * p + 128, :])
            wo.append(t)

        qT = [qk_pool.tile([128, L], BF16, tag=f"qT{m}", name=f"qT{m}") for m in range(4)]
        kT = [qk_pool.tile([128, L], BF16, tag=f"kT{m}", name=f"kT{m}") for m in range(4)]

        PROJ_TILE = {"st0": (st_ps, [128, 512]), "st1": (st_ps, [128, 512]),
                     "outp": (out_ps, [128, 512]), "tr": (tr_ps, [128, 512]),
                     "av0": (av_ps, [128, 512]), "av1": (av_ps, [128, 512])}

        def emit_qk_proj(m, which, n, tag):
            # one psum group (8 matmuls) of the q or k projection
            pool, shape = PROJ_TILE[tag]
            ps = pool.tile(shape, F32, tag=tag, name="proj")
            w_ = wq if which == "q" else wk
            for kd in range(KD):
                nc.tensor.matmul(
                    ps[:], w_[kd][:, 128 * m : 128 * m + 128],
                    xt[kd][:, 512 * n : 512 * n + 512],
                    start=(kd == 0), stop=(kd == KD - 1))
            if which == "q":
                nc.vector.tensor_scalar_add(
                    qT[m][:, 512 * n : 512 * n + 512], ps[:], bq_sb[:, m : m + 1])
            else:
                nc.vector.tensor_copy(kT[m][:, 512 * n : 512 * n + 512], ps[:])

        v_aug = [None] * NS

        def emit_v_proj(s, tag):
            pool, shape = PROJ_TILE[tag]
            ps = pool.tile(shape, F32, tag=tag, name="proj")
            for kd in range(KD):
                nc.tensor.matmul(
                    ps[:], xt[kd][:, 128 * s : 128 * s + 128], wv[kd][:],
                    start=(kd == 0), stop=(kd == KD - 1))
            t = v_pool.tile([128, 520], BF16, tag=f"v{s}", name=f"vaug{s}")
            t3 = t[:].rearrange("p (h e) -> p h e", h=8)
            nc.vector.tensor_copy(t3[:, :, 0:64], ps[:].rearrange("p (h e) -> p h e", h=8))
            nc.vector.memset(t3[:, :, 64:65], 1.0)
            v_aug[s] = t

        # ---- prologue: the 6 groups needed by block (0,0) + q(0,1),
        #      on 6 distinct psum banks ----
        emit_qk_proj(0, "q", 0, "st0")
        emit_qk_proj(0, "q", 1, "st1")
        for n in range(NL):
            emit_qk_proj(0, "k", n, ["outp", "tr", "av0", "av1"][n])

        # qk-projection quanta: block (m,lt) -> list of (pm, which, n),
        # placed in the light g-slots (g=0,1,6,7), each before first use.
        quanta = {
            (0, 1): [(0, "q", 2), (0, "q", 3), (1, "k", 0), (1, "k", 1)],
            (0, 2): [(1, "k", 2), (1, "k", 3), (1, "q", 0), (1, "q", 1)],
            (0, 3): [(1, "q", 2), (1, "q", 3), (2, "k", 0), (2, "k", 1)],
            (1, 0): [(2, "k", 2), (2, "k", 3), (2, "q", 0), (2, "q", 1)],
            (1, 1): [(2, "q", 2), (2, "q", 3), (3, "k", 0), (3, "k", 1)],
            (1, 2): [(3, "k", 2), (3, "k", 3), (3, "q", 0), (3, "q", 1)],
            (1, 3): [(3, "q", 2), (3, "q", 3)],
        }

        # ---- attention blocks ----
        vstage = [vs_pool.tile([128, DH], BF16, tag=f"vs{ls}", name=f"vs{ls}") for ls in range(NS)]

        def emit_av_part(prev, part):
            # one (h2, j) slice of the AV sweep for block `prev`
            pm, plt, ats = prev
            h2, j = divmod(part, 4)
            h = 2 * pm + h2
            avp = av_ps.tile([128, 65], F32, tag=f"av{part % 2}", name="avp")
            for s in range(NS):
                nc.tensor.matmul(
                    avp[:], ats[s][:, 512 * h2 + 128 * j : 512 * h2 + 128 * j + 128],
                    v_aug[s][:, 65 * h : 65 * h + 65],
                    start=(s == 0), stop=(s == NS - 1))
            r = rec_pool.tile([128, 1], F32, tag="rec", name="rec")
            nc.vector.reciprocal(r[:], avp[:, 64:65])
            nc.vector.tensor_scalar_mul(
                vstage[4 * plt + j][:, 64 * h : 64 * h + 64], avp[:, 0:64], r[:])

        def emit_outproj_ls(ls):
            # per-ls out-projection chain: 4 PE transposes -> vt -> 2 psum
            # groups -> osb -> DMA
            vts = []
            for p in range(4):
                tp = tr_ps.tile([128, 128], BF16, tag="tr", name="trp")
                nc.tensor.transpose(tp[:], vstage[ls][:, 128 * p : 128 * p + 128], ident[:])
                vt = vt_pool.tile([128, 128], BF16, tag="vt", name="vt")
                nc.vector.tensor_copy(vt[:], tp[:])
                vts.append(vt)
            osb = osb_pool.tile([128, D], F32, tag="osb", name="osb")
            for d2 in range(2):
                op = out_ps.tile([128, 512], F32, tag="outp", name="outp")
                for p in range(4):
                    nc.tensor.matmul(
                        op[:], vts[p][:], wo[p][:, 512 * d2 : 512 * d2 + 512],
                        start=(p == 0), stop=(p == 3))
                nc.vector.tensor_copy(osb[:, 512 * d2 : 512 * d2 + 512], op[:])
            nc.sync.dma_start(out_d[128 * ls : 128 * ls + 128, :], osb[:])

        outproj_q = []

        def emit_block(m, lt, prev, outproj_new):
            # one (head-pair, l-tile) block: 8 score groups g; each g makes
            # one [128, 1024] psum tile per c2 (both h2 halves, adjacent
            # matmul pair -> concurrent PE row groups) and exps it.
            q = list(quanta.get((m, lt), []))
            vq = list(range(NS)) if (m, lt) == (0, 0) else []
            ats = [None] * NS
            for g in range(8):
                for c2 in range(2):
                    s = 2 * g + c2
                    stc = st_ps.tile([128, 1024], F32, tag=f"st{c2}", name=f"st{c2}")
                    for h2 in range(2):
                        p0 = 64 * h2
                        nc.tensor.matmul(
                            stc[:, 512 * h2 : 512 * h2 + 512],
                            kT[m][p0 : p0 + 64, 128 * s : 128 * s + 128],
                            qT[m][p0 : p0 + 64, 512 * lt : 512 * lt + 512],
                            start=True, stop=True)
                    at = at_pool.tile([128, 1024], BF16, tag=f"at{c2}", name=f"at{c2}")
                    nc.scalar.activation(at[:], stc[:], AF.Exp, scale=SCALE)
                    ats[s] = at
                if 1 <= g <= 4 and prev is not None:
                    emit_av_part(prev, 2 * (g - 1))
                    emit_av_part(prev, 2 * (g - 1) + 1)
                if g == 5:
                    # this block's lt-1 out-proj becomes legal once the AV
                    # parts above (g=1..4) have filled vstage
                    outproj_q.extend(outproj_new)
                    outproj_new = []
                if vq:
                    emit_v_proj(vq.pop(0), ["av0", "av1", "outp", "tr"][g % 4])
                    emit_v_proj(vq.pop(0), ["av1", "outp", "tr", "av0"][g % 4])
                elif g in (0, 5, 6, 7):
                    if q:
                        pm, w_, n = q.pop(0)
                        emit_qk_proj(pm, w_, n, "outp" if g in (0, 6) else "tr")
                    elif outproj_q:
                        emit_outproj_ls(outproj_q.pop(0))
            return ats

        prev = None
        for m in range(4):
            for lt in range(NL):
                new = [4 * (lt - 1) + i for i in range(4)] if (m == 3 and lt > 0) else []
                ats = emit_block(m, lt, prev, new)
                prev = (m, lt, ats)
        for j in range(4):
            emit_av_part(prev, j)        # (h2=0, j)
            emit_av_part(prev, 4 + j)    # (h2=1, j)
            if j == 0:
                for ls in outproj_q:     # leftover from the block loop
                    emit_outproj_ls(ls)
            else:
                emit_outproj_ls(12 + j - 1)
        emit_outproj_ls(15)
        phase1_ctx.close()

    nc.compile()
    return nc


_NC_CACHE = []


def _make_in_maps(inputs):
    x = np.asarray(inputs["x"], dtype=np.float32)
    Wq = np.asarray(inputs["Wq"], dtype=np.float32)
    Wk = np.asarray(inputs["Wk"], dtype=np.float32)
    Wv = np.asarray(inputs["Wv"], dtype=np.float32)
    Wo = np.asarray(inputs["Wo"], dtype=np.float32)
    bq = np.asarray(inputs["bq"], dtype=np.float32)
    bf = ml_dtypes.bfloat16
    in_maps = []
    for c in range(N_CORES):
        b, hh = divmod(c, 2)
        sl = slice(DH * hh, DH * hh + DH)
        in_maps.append({
            "xT": np.ascontiguousarray(x[b].T).astype(bf),
            "wq": np.ascontiguousarray(Wq[:, sl]).astype(bf),
            "wk": np.ascontiguousarray(Wk[:, sl]).astype(bf),
            "wv": np.ascontiguousarray(Wv[:, sl]).astype(bf),
            "wo": np.ascontiguousarray(Wo[sl, :]).astype(bf),
            "bq": np.ascontiguousarray(bq[sl]).reshape(DH, 1).astype(np.float32),
        })
    return in_maps


def kernel(x, Wq, bq, Wk, bk, Wv, bv, Wo, bo):
    x = np.asarray(x, dtype=np.float32)
    Wq = np.asarray(Wq, dtype=np.float32)
    Wk = np.asarray(Wk, dtype=np.float32)
    Wv = np.asarray(Wv, dtype=np.float32)
    Wo = np.asarray(Wo, dtype=np.float32)
    bq = np.asarray(bq, dtype=np.float32)
    bv = np.asarray(bv, dtype=np.float32)
    bo = np.asarray(bo, dtype=np.float32)

    if not _NC_CACHE:
        _NC_CACHE.append(build_attention_nc())
    nc = _NC_CACHE[0]

    in_maps = _make_in_maps(dict(x=x, Wq=Wq, bq=bq, Wk=Wk, Wv=Wv, Wo=Wo))

    res = run_bass_kernel_spmd(nc, in_maps, list(range(N_CORES)))
    parts = [res.results[c]["out"] for c in range(N_CORES)]
    out = np.stack([parts[2 * b] + parts[2 * b + 1] for b in range(B)])
    out += (bv @ Wo + bo)[None, None, :]
    return out.astype(np.float32)


# revision 26
# speedup vs baseline: 1.3938x; 1.0304x over previous
"""Multi-head attention layer (B=4, L=2048, D=1024, H=16) on 8 TRN2 NeuronCores.

Sharding: core c handles batch b = c//2 and heads [8*(c%2), 8*(c%2)+8) —
batch-parallel x tensor-parallel over heads.  Host sums the two partial
outputs per batch and adds bv@Wo + bo (bk drops exactly by softmax shift
invariance).

Per-core dataflow (all matmul inputs bf16, fp32 accumulation):
  qT/kT = Wq/Wk_slice as stationary against xT  -> [512, 2048] (e on partitions)
  v     = x @ Wv_slice (+ones col per head)      -> [2048, 8*65]
  scores: per (g, c2) one psum tile [128 s, (h2, 512 l)] written by an
  adjacent pair of K=64 matmuls targeting PE row groups 0-63/64-127 (they
  run concurrently).  A = exp on ACT, FD=1024 per instruction.
  AV: per (h2, j) part: 16-matmul accumulation (A^T chunk stationary,
  v_aug moving, N=65); ones col -> softmax denom in col 64; normalize via
  reciprocal+tensor_scalar (DVE); PE-transpose V into VT;
  out_partial = VT.T @ Wo_slice.

Scheduling: blocks pair-major (m outer, lt inner), one block = 8 score
groups g feeding 16 ACT exps of 1147ns — the ACT engine is the roofline
(~295us busy) and everything else is paced to hide under it.  Per block:
score pairs ping-pong 2 psum tags; the previous block's AV runs as 8
parts in g=2..5; projection quanta (2-bank rotation outp/tr) fill
g=0,1,6,7 of m<3 blocks; out-proj per-ls chains fill those slots in m=3
blocks.  Prologue is 6 proj groups on 6 distinct psum banks fed by
column-quartered xT DMAs.
"""

import sys
from contextlib import ExitStack

for _p in ("/opt/trn_rl_repo", "/root/.axon_site/_ro/trn_rl_repo"):
    if _p not in sys.path:
        sys.path.append(_p)

import numpy as np
import ml_dtypes

import concourse.bass as bass
import concourse.mybir as mybir
import concourse.tile as tile
from concourse import bacc
from concourse.bass_utils import run_bass_kernel_spmd
from concourse.masks import make_identity

BF16 = mybir.dt.bfloat16
F32 = mybir.dt.float32
AF = mybir.ActivationFunctionType

B, L, D = 4, 2048, 1024
N_CORES = 8
DH = 512          # per-core head dims (8 heads x 64)
E = 64
SCALE = 0.125     # 1/sqrt(E)

KD = D // 128     # 8 contraction chunks for projections
NL = L // 512     # 4 l-tiles
NS = L // 128     # 16 s-chunks / l-subs


def build_attention_nc():
    nc = bacc.Bacc("TRN2", target_bir_lowering=False, debug=False)

    # Host pre-arranges everything into [128, N] wide-tile layouts so each
    # input is a single contiguous DMA (dma_start issue costs ~0.6us each).
    # xT: [p, 4096*q + 512*kd + c] = x.T[128*kd + p, 512*q + c]
    # wq/wk/wv: [p, 512*kd + c];  wo: [p, 1024*pq + c];  bq: [p, m]
    xT_d = nc.dram_tensor("xT", [128, KD * L], BF16, kind="ExternalInput").ap()
    wq_d = nc.dram_tensor("wq", [128, KD * DH], BF16, kind="ExternalInput").ap()
    wk_d = nc.dram_tensor("wk", [128, KD * DH], BF16, kind="ExternalInput").ap()
    wv_d = nc.dram_tensor("wv", [128, KD * DH], BF16, kind="ExternalInput").ap()
    wo_d = nc.dram_tensor("wo", [128, 4 * D], BF16, kind="ExternalInput").ap()
    bq_d = nc.dram_tensor("bq", [128, 4], F32, kind="ExternalInput").ap()
    out_d = nc.dram_tensor("out", [L, D], F32, kind="ExternalOutput").ap()

    with tile.TileContext(nc) as tc, ExitStack() as ctx:
        const_pool = ctx.enter_context(tc.tile_pool(name="const", bufs=1))
        w_pool = ctx.enter_context(tc.tile_pool(name="w", bufs=1))
        qk_pool = ctx.enter_context(tc.tile_pool(name="qk", bufs=1))
        v_pool = ctx.enter_context(tc.tile_pool(name="v", bufs=1))
        at_pool = ctx.enter_context(tc.tile_pool(name="at", bufs=14))
        vs_pool = ctx.enter_context(tc.tile_pool(name="vs", bufs=1))
        vt_pool = ctx.enter_context(tc.tile_pool(name="vt", bufs=12))
        rec_pool = ctx.enter_context(tc.tile_pool(name="rec", bufs=8))
        osb_pool = ctx.enter_context(tc.tile_pool(name="osb", bufs=2))

        st_ps = ctx.enter_context(tc.tile_pool(name="st_ps", bufs=1, space="PSUM"))
        av_ps = ctx.enter_context(tc.tile_pool(name="av_ps", bufs=1, space="PSUM"))
        tr_ps = ctx.enter_context(tc.tile_pool(name="tr_ps", bufs=1, space="PSUM"))
        out_ps = ctx.enter_context(tc.tile_pool(name="out_ps", bufs=1, space="PSUM"))
        phase1_ctx = ExitStack()
        xt_pool = phase1_ctx.enter_context(tc.tile_pool(name="xt", bufs=1))

        ident = const_pool.tile([128, 128], BF16, tag="ident", name="ident")
        make_identity(nc, ident[:])
        bq_sb = const_pool.tile([128, 4], F32, tag="bq", name="bq_sb")
        nc.sync.dma_start(bq_sb[:], bq_d[:, :])

        # Consolidated DMAs (each dma_start costs ~0.6us of issue time on the
        # Sync queue): one wide tile per tensor.  Order: wq/wk, xT by column
        # quarters, wv, wo — so the prologue projections start early.
        wq_sb = xt_pool.tile([128, KD * DH], BF16, tag="wq", name="wq_sb")
        wk_sb = xt_pool.tile([128, KD * DH], BF16, tag="wk", name="wk_sb")
        wv_sb = xt_pool.tile([128, KD * DH], BF16, tag="wv", name="wv_sb")
        xt_sb = xt_pool.tile([128, KD * L], BF16, tag="xt", name="xt_sb")
        wo_sb = w_pool.tile([128, 4 * D], BF16, tag="wo", name="wo_sb")
        nc.sync.dma_start(wq_sb[:], wq_d[:, :])
        nc.sync.dma_start(wk_sb[:], wk_d[:, :])
        for q4 in range(4):
            nc.sync.dma_start(
                xt_sb[:, 4096 * q4 : 4096 * q4 + 4096],
                xT_d[:, 4096 * q4 : 4096 * q4 + 4096])
        nc.sync.dma_start(wv_sb[:], wv_d[:, :])
        nc.sync.dma_start(wo_sb[:], wo_d[:, :])


        qT = [qk_pool.tile([128, L], BF16, tag=f"qT{m}", name=f"qT{m}") for m in range(4)]
        kT = [qk_pool.tile([128, L], BF16, tag=f"kT{m}", name=f"kT{m}") for m in range(4)]

        PROJ_TILE = {"st0": (st_ps, [128, 512]), "st1": (st_ps, [128, 512]),
                     "outp": (out_ps, [128, 512]), "tr": (tr_ps, [128, 512]),
                     "av0": (av_ps, [128, 512]), "av1": (av_ps, [128, 512])}

        def emit_qk_proj(m, which, n, tag):
            # one psum group (8 matmuls) of the q or k projection
            pool, shape = PROJ_TILE[tag]
            ps = pool.tile(shape, F32, tag=tag, name="proj")
            w_sb = wq_sb if which == "q" else wk_sb
            for kd in range(KD):
                nc.tensor.matmul(
                    ps[:], w_sb[:, DH * kd + 128 * m : DH * kd + 128 * m + 128],
                    xt_sb[:, 4096 * n + 512 * kd : 4096 * n + 512 * kd + 512],
                    start=(kd == 0), stop=(kd == KD - 1))
            if which == "q":
                nc.vector.tensor_scalar_add(
                    qT[m][:, 512 * n : 512 * n + 512], ps[:], bq_sb[:, m : m + 1])
            else:
                nc.vector.tensor_copy(kT[m][:, 512 * n : 512 * n + 512], ps[:])

        v_aug = [None] * NS

        def emit_v_proj(s, tag):
            pool, shape = PROJ_TILE[tag]
            ps = pool.tile(shape, F32, tag=tag, name="proj")
            for kd in range(KD):
                nc.tensor.matmul(
                    ps[:], xt_sb[:, 4096 * (s // 4) + 512 * kd + 128 * (s % 4)
                           : 4096 * (s // 4) + 512 * kd + 128 * (s % 4) + 128],
                    wv_sb[:, DH * kd : DH * kd + DH],
                    start=(kd == 0), stop=(kd == KD - 1))
            t = v_pool.tile([128, 520], BF16, tag=f"v{s}", name=f"vaug{s}")
            t3 = t[:].rearrange("p (h e) -> p h e", h=8)
            nc.vector.tensor_copy(t3[:, :, 0:64], ps[:].rearrange("p (h e) -> p h e", h=8))
            nc.vector.memset(t3[:, :, 64:65], 1.0)
            v_aug[s] = t

        # ---- prologue: the 6 groups needed by block (0,0) + q(0,1),
        #      on 6 distinct psum banks ----
        emit_qk_proj(0, "q", 0, "st0")
        emit_qk_proj(0, "q", 1, "st1")
        for n in range(NL):
            emit_qk_proj(0, "k", n, ["outp", "tr", "av0", "av1"][n])

        # qk-projection quanta: block (m,lt) -> list of (pm, which, n),
        # placed in the light g-slots (g=0,1,6,7), each before first use.
        quanta = {
            (0, 1): [(0, "q", 2), (0, "q", 3), (1, "k", 0), (1, "k", 1)],
            (0, 2): [(1, "k", 2), (1, "k", 3), (1, "q", 0), (1, "q", 1)],
            (0, 3): [(1, "q", 2), (1, "q", 3), (2, "k", 0), (2, "k", 1)],
            (1, 0): [(2, "k", 2), (2, "k", 3), (2, "q", 0), (2, "q", 1)],
            (1, 1): [(2, "q", 2), (2, "q", 3), (3, "k", 0), (3, "k", 1)],
            (1, 2): [(3, "k", 2), (3, "k", 3), (3, "q", 0), (3, "q", 1)],
            (1, 3): [(3, "q", 2), (3, "q", 3)],
        }

        # ---- attention blocks ----
        vstage = [vs_pool.tile([128, DH], BF16, tag=f"vs{ls}", name=f"vs{ls}") for ls in range(NS)]

        def emit_av_part(prev, part):
            # one (h2, j) slice of the AV sweep for block `prev`
            pm, plt, ats = prev
            h2, j = divmod(part, 4)
            h = 2 * pm + h2
            avp = av_ps.tile([128, 65], F32, tag=f"av{part % 2}", name="avp")
            for s in range(NS):
                nc.tensor.matmul(
                    avp[:], ats[s][:, 512 * h2 + 128 * j : 512 * h2 + 128 * j + 128],
                    v_aug[s][:, 65 * h : 65 * h + 65],
                    start=(s == 0), stop=(s == NS - 1))
            r = rec_pool.tile([128, 1], F32, tag="rec", name="rec")
            nc.vector.reciprocal(r[:], avp[:, 64:65])
            nc.vector.tensor_scalar_mul(
                vstage[4 * plt + j][:, 64 * h : 64 * h + 64], avp[:, 0:64], r[:])

        def emit_outproj_tr(ls, tr_tag="tr"):
            # 4 PE transposes of a vstage row-chunk -> vt tiles
            vts = []
            for p in range(4):
                pool = PROJ_TILE[tr_tag][0]
                tp = pool.tile([128, 128], BF16, tag=tr_tag, name="trp")
                nc.tensor.transpose(tp[:], vstage[ls][:, 128 * p : 128 * p + 128], ident[:])
                vt = vt_pool.tile([128, 128], BF16, tag="vt", name="vt")
                nc.vector.tensor_copy(vt[:], tp[:])
                vts.append(vt)
            return vts

        def emit_outproj_mm(ls, vts, op_tag="outp"):
            # 2 psum groups -> osb -> DMA
            osb = osb_pool.tile([128, D], F32, tag="osb", name="osb")
            for d2 in range(2):
                pool = PROJ_TILE[op_tag][0]
                op = pool.tile([128, 512], F32, tag=op_tag, name="outp")
                for p in range(4):
                    nc.tensor.matmul(
                        op[:], vts[p][:],
                        wo_sb[:, D * p + 512 * d2 : D * p + 512 * d2 + 512],
                        start=(p == 0), stop=(p == 3))
                nc.vector.tensor_copy(osb[:, 512 * d2 : 512 * d2 + 512], op[:])
            nc.sync.dma_start(out_d[128 * ls : 128 * ls + 128, :], osb[:])

        def emit_outproj_ls(ls):
            emit_outproj_mm(ls, emit_outproj_tr(ls))

        outproj_q = []
        qk_state = {"cur": None, "ntag": 0}

        def emit_qk_half(q):
            # one 4-matmul half of a qk-projection quantum; keeps the PE
            # FIFO detour between score pairs under ~1us
            if qk_state["cur"] is None:
                if not q:
                    return
                m_, w_, n_ = q.pop(0)
                tag = ("outp", "tr")[qk_state["ntag"] % 2]
                qk_state["ntag"] += 1
                pool, shape = PROJ_TILE[tag]
                ps = pool.tile(shape, F32, tag=tag, name="proj")
                kds = range(0, 4)
                qk_state["cur"] = (ps, m_, w_, n_)
            else:
                ps, m_, w_, n_ = qk_state["cur"]
                kds = range(4, KD)
            w_sb = wq_sb if w_ == "q" else wk_sb
            for kd in kds:
                nc.tensor.matmul(
                    ps[:], w_sb[:, DH * kd + 128 * m_ : DH * kd + 128 * m_ + 128],
                    xt_sb[:, 4096 * n_ + 512 * kd : 4096 * n_ + 512 * kd + 512],
                    start=(kd == 0), stop=(kd == KD - 1))
            if kds.stop == KD:
                if w_ == "q":
                    nc.vector.tensor_scalar_add(
                        qT[m_][:, 512 * n_ : 512 * n_ + 512], ps[:], bq_sb[:, m_ : m_ + 1])
                else:
                    nc.vector.tensor_copy(kT[m_][:, 512 * n_ : 512 * n_ + 512], ps[:])
                qk_state["cur"] = None

        def emit_block(m, lt, prev, outproj_new):
            # one (head-pair, l-tile) block: 8 score groups g; each g makes
            # one [128, 1024] psum tile per c2 (both h2 halves, adjacent
            # matmul pair -> concurrent PE row groups) and exps it.
            q = list(quanta.get((m, lt), []))
            vq = list(range(NS)) if (m, lt) == (0, 0) else []
            ats = [None] * NS
            for g in range(8):
                for c2 in range(2):
                    s = 2 * g + c2
                    stc = st_ps.tile([128, 1024], F32, tag=f"st{c2}", name=f"st{c2}")
                    for h2 in range(2):
                        p0 = 64 * h2
                        nc.tensor.matmul(
                            stc[:, 512 * h2 : 512 * h2 + 512],
                            kT[m][p0 : p0 + 64, 128 * s : 128 * s + 128],
                            qT[m][p0 : p0 + 64, 512 * lt : 512 * lt + 512],
                            start=True, stop=True)
                    at = at_pool.tile([128, 1024], BF16, tag=f"at{c2}", name=f"at{c2}")
                    nc.scalar.activation(at[:], stc[:], AF.Exp, scale=SCALE)
                    ats[s] = at
                if 1 <= g <= 4 and prev is not None:
                    emit_av_part(prev, 2 * (g - 1))
                    emit_av_part(prev, 2 * (g - 1) + 1)
                if g == 5:
                    # this block's lt-1 out-proj becomes legal once the AV
                    # parts above (g=1..4) have filled vstage
                    outproj_q.extend(outproj_new)
                    outproj_new = []
                if vq:
                    emit_v_proj(vq.pop(0), ["av0", "av1", "outp", "tr"][g % 4])
                    emit_v_proj(vq.pop(0), ["av1", "outp", "tr", "av0"][g % 4])
                elif q or qk_state["cur"] is not None:
                    emit_qk_half(q)
                elif g in (0, 5, 6, 7) and outproj_q:
                    emit_outproj_ls(outproj_q.pop(0))
            return ats

        prev = None
        for m in range(4):
            for lt in range(NL):
                new = [4 * (lt - 1) + i for i in range(4)] if (m == 3 and lt > 0) else []
                ats = emit_block(m, lt, prev, new)
                prev = (m, lt, ats)
        emit_av_part(prev, 0)
        emit_av_part(prev, 4)
        for ls in outproj_q:             # leftover from the block loop
            emit_outproj_ls(ls)
        emit_av_part(prev, 1)
        emit_av_part(prev, 5)
        emit_outproj_ls(12)
        emit_av_part(prev, 2)
        emit_av_part(prev, 6)
        emit_av_part(prev, 3)
        emit_av_part(prev, 7)
        # av banks are free now: run two out-proj pipelines, stage-interleaved
        vts13 = emit_outproj_tr(13, "av0")
        vts14 = emit_outproj_tr(14, "tr")
        emit_outproj_mm(13, vts13, "av1")
        vts15 = emit_outproj_tr(15, "av0")
        emit_outproj_mm(14, vts14, "outp")
        emit_outproj_mm(15, vts15, "av1")
        phase1_ctx.close()

    nc.compile()
    return nc


_NC_CACHE = []


def _make_in_maps(inputs):
    x = np.asarray(inputs["x"], dtype=np.float32)
    Wq = np.asarray(inputs["Wq"], dtype=np.float32)
    Wk = np.asarray(inputs["Wk"], dtype=np.float32)
    Wv = np.asarray(inputs["Wv"], dtype=np.float32)
    Wo = np.asarray(inputs["Wo"], dtype=np.float32)
    bq = np.asarray(inputs["bq"], dtype=np.float32)
    bf = ml_dtypes.bfloat16

    def wide_kd(w):  # [1024, C] -> [128, 8*C], kd-chunks side by side
        c = w.shape[1]
        return np.ascontiguousarray(
            w.reshape(KD, 128, c).transpose(1, 0, 2).reshape(128, KD * c))

    in_maps = []
    for c in range(N_CORES):
        b, hh = divmod(c, 2)
        sl = slice(DH * hh, DH * hh + DH)
        xT = x[b].T  # [1024, 2048]
        # [p, 4096*q + 512*kd + col] = xT[128*kd + p, 512*q + col]
        xT_w = xT.reshape(KD, 128, 4, 512).transpose(1, 2, 0, 3).reshape(128, KD * L)
        wo_w = Wo[sl, :].reshape(4, 128, D).transpose(1, 0, 2).reshape(128, 4 * D)
        in_maps.append({
            "xT": np.ascontiguousarray(xT_w).astype(bf),
            "wq": wide_kd(Wq[:, sl]).astype(bf),
            "wk": wide_kd(Wk[:, sl]).astype(bf),
            "wv": wide_kd(Wv[:, sl]).astype(bf),
            "wo": np.ascontiguousarray(wo_w).astype(bf),
            "bq": np.ascontiguousarray(bq[sl].reshape(4, 128).T).astype(np.float32),
        })
    return in_maps


def kernel(x, Wq, bq, Wk, bk, Wv, bv, Wo, bo):
    x = np.asarray(x, dtype=np.float32)
    Wq = np.asarray(Wq, dtype=np.float32)
    Wk = np.asarray(Wk, dtype=np.float32)
    Wv = np.asarray(Wv, dtype=np.float32)
    Wo = np.asarray(Wo, dtype=np.float32)
    bq = np.asarray(bq, dtype=np.float32)
    bv = np.asarray(bv, dtype=np.float32)
    bo = np.asarray(bo, dtype=np.float32)

    if not _NC_CACHE:
        _NC_CACHE.append(build_attention_nc())
    nc = _NC_CACHE[0]

    in_maps = _make_in_maps(dict(x=x, Wq=Wq, bq=bq, Wk=Wk, Wv=Wv, Wo=Wo))

    res = run_bass_kernel_spmd(nc, in_maps, list(range(N_CORES)))
    parts = [res.results[c]["out"] for c in range(N_CORES)]
    out = np.stack([parts[2 * b] + parts[2 * b + 1] for b in range(B)])
    out += (bv @ Wo + bo)[None, None, :]
    return out.astype(np.float32)
